# revision 1
# baseline (speedup 1.0000x reference)
"""Trainium2 Bass kernel for nn_GAT_12232066859439.

3-layer GAT + 6-head MLP readout. Strategy:
  - GAT layers computed redundantly on all 8 cores (cheap: the N^2 attention
    collapses algebraically -- e2 has only g=N/f distinct rows -- and
    masked softmax reduces to adj * exp(s) / rowsum, so no [N,N] softmax
    materialization is needed).
  - The 402MB l1w matvec (the memory-bound bulk) is sharded 192 rows/core;
    t1 is AllGathered, l2/l3 computed redundantly; output taken from core 0.
  - fp16 data with fp32 PSUM accumulation for the big streams (1 cyc/row on
    the PE vs 4 for fp32, and half the HBM traffic).
"""
import os
import sys

sys.path.insert(0, "/opt/trn_rl_repo")

import numpy as np

import concourse.bacc as bacc
import concourse.bass as bass
import concourse.tile as tile
from concourse import mybir
from concourse.bass_utils import run_bass_kernel_spmd

F32 = mybir.dt.float32
F16 = mybir.dt.float16
U8 = mybir.dt.uint8
AF = mybir.ActivationFunctionType
ALU = mybir.AluOpType

P = 128
N = 1024
NCORES = 8
NCH = N // P  # 8 row-chunks
# (Fin, F, g) per GAT layer
LAYERS = [(512, 128, 8), (128, 64, 16), (64, 64, 16)]
RSHARD = 1536 // NCORES  # 192 l1 rows per core
KCH = 65536 // P         # 512 contraction chunks for l1
SLAB = 32                # k-chunks per A-slab DMA
D2 = 256                 # l2 contraction size

NP_GAT = np.float16

_CACHE = {}


def ts(i, n):
    return slice(i * n, (i + 1) * n)


def _build():
    nc = bacc.Bacc("TRN2", target_bir_lowering=False, debug=False,
                   num_devices=NCORES)
    DT = F16  # GAT compute dtype (PSUM accumulation is fp32 regardless)

    # ---- inputs (replicated unless noted) ----
    xT_d = nc.dram_tensor("xT", [512, N], DT, kind="ExternalInput")
    adjT_d = nc.dram_tensor("adjT", [N, N], F16, kind="ExternalInput")
    ident_d = nc.dram_tensor("ident", [P, P], F32, kind="ExternalInput")
    W_d = [nc.dram_tensor(f"W{l+1}", [LAYERS[l][0], LAYERS[l][1]], DT,
                          kind="ExternalInput") for l in range(3)]
    aF_d = [nc.dram_tensor(f"aF{l+1}", [P, LAYERS[l][1]], DT,
                           kind="ExternalInput") for l in range(3)]
    b_d = [nc.dram_tensor(f"b{l+1}", [P, LAYERS[l][1]], F32,
                          kind="ExternalInput") for l in range(3)]
    bT_d = [nc.dram_tensor(f"bT{l+1}", [P, 1], F32,
                           kind="ExternalInput") for l in range(3)]
    wsel_d = [nc.dram_tensor(f"wsel{l+1}", [N, LAYERS[l][2]], DT,
                             kind="ExternalInput") for l in range(3)]
    A_d = nc.dram_tensor("A", [P, KCH, RSHARD], F16, kind="ExternalInput")  # per-core
    l1bs_d = nc.dram_tensor("l1bs", [1, RSHARD], F32, kind="ExternalInput")  # per-core
    l1bn_d = nc.dram_tensor("l1bn", [12, P], F32, kind="ExternalInput")
    L2T_d = nc.dram_tensor("L2T", [D2, 6 * P], F16, kind="ExternalInput")
    l2bT_d = nc.dram_tensor("l2bT", [P, 6], F32, kind="ExternalInput")
    l3wT_d = nc.dram_tensor("l3wT", [P, 6], F32, kind="ExternalInput")
    l3b_d = nc.dram_tensor("l3b", [1, 6], F32, kind="ExternalInput")

    out_d = nc.dram_tensor("out", [6, 1], F32, kind="ExternalOutput")

    with tile.TileContext(nc) as tc:
        with tc.tile_pool(name="const", bufs=1) as const, \
             tc.tile_pool(name="work", bufs=1) as work, \
             tc.tile_pool(name="ps", bufs=1, space="PSUM") as psp, \
             tc.tile_pool(name="dram", bufs=1, space="DRAM") as dram:

            # ---- constant loads ----
            ident = const.tile([P, P], F32)
            nc.sync.dma_start(ident[:], ident_d[:])

            xT = const.tile([P, 4 * N], DT, name="xT_sb")
            for kc in range(4):
                nc.sync.dma_start(xT[:, ts(kc, N)], xT_d[ts(kc, P), :])

            W1_sb = const.tile([P, 4 * LAYERS[0][1]], DT, name="W_sb0")
            for kc in range(4):
                nc.sync.dma_start(W1_sb[:, ts(kc, LAYERS[0][1])],
                                  W_d[0][ts(kc, P), :])

            adjT = const.tile([P, NCH * N], DT, name="adjT_c")
            for kc in range(NCH):
                nc.sync.dma_start(adjT[:, ts(kc, N)], adjT_d[ts(kc, P), :])

            W_sb, aF_sb, b_sb, bT_sb, wsel_sb = [], [], [], [], []
            for l, (Fin, F, g) in enumerate(LAYERS):
                nk = max(1, Fin // P)
                if l == 0:
                    w = W1_sb
                else:
                    w = const.tile([P, nk * F], DT, name=f"W_sb{l}")
                    for kc in range(nk):
                        kp = min(P, Fin)
                        nc.sync.dma_start(w[:kp, ts(kc, F)],
                                          W_d[l][ts(kc, kp), :])
                W_sb.append(w)
                af = const.tile([P, F], DT, name=f"aF_sb{l}")
                nc.sync.dma_start(af[:], aF_d[l][:])
                aF_sb.append(af)
                bb = const.tile([P, F], F32, name=f"b_sb{l}")
                nc.sync.dma_start(bb[:], b_d[l][:])
                b_sb.append(bb)
                bt = const.tile([P, 1], F32, name=f"bT_sb{l}")
                nc.sync.dma_start(bt[:], bT_d[l][:])
                bT_sb.append(bt)
                wsl = const.tile([P, NCH * g], DT, name=f"wsel_sb{l}")
                for m in range(NCH):
                    nc.sync.dma_start(wsl[:, ts(m, g)], wsel_d[l][ts(m, P), :])
                wsel_sb.append(wsl)

            l1bs = const.tile([1, RSHARD], F32)
            nc.sync.dma_start(l1bs[:], l1bs_d[:])
            l1bn = const.tile([12, P], F32)
            nc.sync.dma_start(l1bn[:], l1bn_d[:])
            L2T = const.tile([P, 2 * 6 * P], F16, name="L2T_sb")
            for k in range(2):
                nc.sync.dma_start(L2T[:, ts(k, 6 * P)], L2T_d[ts(k, P), :])
            l2bT = const.tile([P, 6], F32)
            nc.sync.dma_start(l2bT[:], l2bT_d[:])
            l3wT = const.tile([P, 6], F32)
            nc.sync.dma_start(l3wT[:], l3wT_d[:])
            l3b = const.tile([1, 6], F32)
            nc.sync.dma_start(l3b[:], l3b_d[:])

            ones_row = const.tile([1, P], F32, name="ones_row")
            nc.vector.memset(ones_row[:], 1.0)

            h3v = dram.tile([512, P], F32, name="h3v")

            # ---- GAT layers ----
            prev_hT = None  # [F_prev, N] sbuf tile for layers 2,3
            for l, (Fin, F, g) in enumerate(LAYERS):
                nk = max(1, Fin // P)
                kp = min(P, Fin)

                haug = work.tile([P, NCH * (F + 1)], DT, name=f"haug{l}",
                                 tag="haug", bufs=2)
                e1 = work.tile([P, NCH], F32, name=f"e1_{l}", tag="e1", bufs=2)

                for m in range(NCH):
                    hp = psp.tile([P, F], F32, name=f"hp{l}_{m}", tag="hproj",
                                  bufs=1)
                    for kc in range(nk):
                        if l == 0:
                            lhsT = xT[:, kc * N + m * P: kc * N + (m + 1) * P]
                        else:
                            lhsT = prev_hT[:kp, ts(m, P)]
                        nc.tensor.matmul(hp[:], lhsT, W_sb[l][:kp, ts(kc, F)],
                                         start=(kc == 0), stop=(kc == nk - 1))
                    col = m * (F + 1)
                    nc.scalar.activation(haug[:, col: col + F], hp[:],
                                         AF.Copy)
                    nc.vector.memset(haug[:, col + F: col + F + 1], 1.0)
                haug_v = haug.rearrange("p (m f) -> p m f", f=F + 1)[:, :, 0:F]
                tmpa = work.tile([P, NCH * F], F32, name=f"e1t{l}",
                                 tag="e1tmp", bufs=2)
                nc.vector.tensor_mul(
                    tmpa.rearrange("p (m f) -> p m f", f=F), haug_v,
                    aF_sb[l][:].broadcast_to([P, F, NCH]).rearrange(
                        "p f m -> p m f"))
                nc.vector.reduce_sum(e1[:], tmpa.rearrange(
                    "p (m f) -> p m f", f=F), axis=mybir.AxisListType.X)

                # E2T[v, u] = sum_i wsel[i, u] * hproj[i, v]
                e2ps = psp.tile([F, g], F32, name=f"e2ps{l}", tag="small",
                                bufs=1)
                for m in range(NCH):
                    col = m * (F + 1)
                    nc.tensor.matmul(e2ps[:], haug[:, col: col + F],
                                     wsel_sb[l][:, ts(m, g)],
                                     start=(m == 0), stop=(m == NCH - 1))
                e2T = work.tile([P, g], F32, name=f"e2T{l}", tag="e2T", bufs=2)
                nc.vector.tensor_copy(e2T[:F, :], e2ps[:])
                if F == 64:
                    nc.sync.dma_start(e2T[64:128, :], e2T[0:64, :])

                # e_all[:, kc*g + u] = exp(lrelu(e1[:, kc] + E2T[:, u]))
                s_scr = work.tile([P, NCH * g], F32, name=f"sscr{l}",
                                  tag="sscr", bufs=2)
                nc.vector.tensor_add(
                    s_scr.rearrange("p (k u) -> p k u", u=g),
                    e1[:].broadcast_to([P, NCH, g]),
                    e2T[:, 0:g].broadcast_to([P, g, NCH]).rearrange(
                        "p u k -> p k u"))
                nc.vector.scalar_tensor_tensor(s_scr[:], s_scr[:], 0.2,
                                               s_scr[:], ALU.mult, ALU.max)
                e_all = work.tile([P, NCH * g], DT, name=f"eall{l}",
                                  tag="eall", bufs=2)
                exp_inst = nc.scalar.activation(e_all[:], s_scr[:], AF.Exp)
                if l == 0:
                    anchor_inst = exp_inst

                # attention: numer[i, :] = sum_j adj[i,j] E_{u(i)}[j] haug[j, :]
                adjT_v = adjT.rearrange("p (k j) -> p k j", j=N)
                e_all_v = e_all.rearrange("p (k u) -> p k u", u=g)
                if l < 2:
                    new_hT = work.tile([P, N], DT, name=f"hT{l}",
                                       tag="hT", bufs=2)
                for m in range(NCH):
                    w = work.tile([P, NCH * P], DT, name=f"wun{l}_{m}",
                                  tag="wun", bufs=3)
                    wv = w.rearrange("p (k j) -> p k j", j=P)
                    for hk in range(2):
                        ks = slice(hk * 4, hk * 4 + 4)
                        if F == P:
                            nc.vector.tensor_mul(
                                wv[:, ks, :],
                                adjT_v[:, ks, m * P: (m + 1) * P],
                                e_all[:, m: NCH * g: g][:, ks].broadcast_to(
                                    [P, 4, P]))
                        else:
                            nc.vector.tensor_mul(
                                wv[:, ks, :].rearrange(
                                    "p k (uu v) -> p k uu v", v=64),
                                adjT_v[:, ks, m * P: (m + 1) * P].rearrange(
                                    "p k (uu v) -> p k uu v", v=64),
                                e_all_v[:, ks, 2 * m: 2 * m + 2].broadcast_to(
                                    [P, 4, 2, 64]))
                    nps = psp.tile([P, F + 1], F32, name=f"nps{l}_{m}",
                                   tag="numer", bufs=3)
                    for kc in range(NCH):
                        nc.tensor.matmul(nps[:], w[:, ts(kc, P)],
                                         haug[:, kc * (F + 1): (kc + 1) * (F + 1)],
                                         start=(kc == 0), stop=(kc == NCH - 1))
                    rd = work.tile([P, 1], F32, name=f"rd{l}_{m}", tag="rd",
                                   bufs=3)
                    nc.vector.reciprocal(rd[:], nps[:, F: F + 1])
                    y = work.tile([P, F], F32, name=f"y{l}_{m}", tag="y",
                                  bufs=3)
                    nc.scalar.activation(y[:], nps[:, 0:F], AF.Relu,
                                         scale=rd[:])
                    hn = work.tile([P, F], F32, name=f"hn{l}_{m}", tag="hn",
                                   bufs=3)
                    nc.vector.tensor_add(hn[:], y[:], b_sb[l][:])
                    nc.vector.tensor_scalar_max(hn[:], hn[:], 0.0)
                    if l < 2:
                        tp = psp.tile([F, P], F32, name=f"tp{l}_{m}",
                                      tag="tp", bufs=2)
                        nc.tensor.transpose(tp[:], hn[:], ident[:])
                        nc.vector.tensor_copy(new_hT[:F, ts(m, P)], tp[:])
                    else:
                        nc.sync.dma_start(h3v[ts(m, 64), :], hn[:])
                if l < 2:
                    prev_hT = new_hT

            # ---- MLP head ----
            # xf_sb[p, c] = xf[c*128 + p]
            xf = work.tile([P, KCH], F16, name="xf_sb")
            for t in range(4):
                v = work.tile([P, P], F32, name=f"xfin{t}", tag="xfin", bufs=2)
                nc.sync.dma_start(v[:], h3v[ts(t, P), :])
                tp = psp.tile([P, P], F32, name=f"xtp{t}", tag="tp", bufs=2)
                nc.tensor.transpose(tp[:], v[:], ident[:])
                nc.vector.tensor_copy(xf[:, ts(t, P)], tp[:])

            t1ps = psp.tile([2, 2 * RSHARD], F32, name="t1ps", tag="t1ps",
                            bufs=1)
            nslab = KCH // SLAB
            npair = KCH // 2
            gwarm = work.tile([1, 8], F32, name="gwarm")
            for si in range(nslab):
                a_sb = work.tile([P, SLAB * RSHARD], F16, name=f"aslab{si}",
                                 tag="aslab", bufs=8)
                nc.sync.dma_start(a_sb[:], A_d[:, ts(si, SLAB), :])
                nc.gpsimd.memset(gwarm[:], float(si))
                for pp in range(SLAB // 2):
                    p_ = si * (SLAB // 2) + pp
                    nc.tensor.matmul(t1ps[:], xf[:, 2 * p_: 2 * p_ + 2],
                                     a_sb[:, ts(pp, 2 * RSHARD)],
                                     start=(p_ == 0), stop=(p_ == npair - 1))
            t1c = work.tile([2, 2 * RSHARD], F32, name="t1c")
            nc.vector.tensor_copy(t1c[:], t1ps[:])
            t1b = work.tile([1, RSHARD], F32, name="t1b")
            nc.sync.dma_start(t1b[:], t1c[1:2, RSHARD:2 * RSHARD])
            t1a = work.tile([1, RSHARD], F32, name="t1a")
            nc.vector.tensor_add(t1a[:], t1c[0:1, 0:RSHARD], t1b[:])

            ag_in = dram.tile([1, RSHARD], F32, name="ag_in")
            ag_out = dram.tile([NCORES, RSHARD], F32, name="ag_out",
                               addr_space="Shared")
            nc.sync.dma_start(ag_in[:], t1a[:])
            nc.gpsimd.collective_compute(
                "AllGather", ALU.bypass,
                replica_groups=[list(range(NCORES))],
                ins=[ag_in.opt()], outs=[ag_out.opt()])

            t1n = work.tile([12, P], F32, name="t1n")
            nc.sync.dma_start(
                t1n[:], ag_out.rearrange("a b -> (a b)").rearrange(
                    "(a b) -> a b", b=P))
            nc.vector.tensor_add(t1n[:], t1n[:], l1bn[:])
            nc.vector.tensor_scalar_max(t1n[:], t1n[:], 0.0)
            t1tp = psp.tile([P, 12], F32, name="t1tp", tag="small", bufs=1)
            nc.tensor.transpose(t1tp[:], t1n[:], ident[0:12, 0:12])
            t1T = work.tile([P, 12], F16, name="t1T")
            nc.vector.tensor_copy(t1T[:], t1tp[:])

            t2 = work.tile([P, 6], F32, name="t2_sb")
            for h in range(6):
                t2ps = psp.tile([P, 1], F32, name=f"t2ps{h}", tag="small",
                                bufs=1)
                for k in range(2):
                    nc.tensor.matmul(t2ps[:],
                                     L2T[:, k * 6 * P + h * P: k * 6 * P + (h + 1) * P],
                                     t1T[:, 2 * h + k: 2 * h + k + 1],
                                     start=(k == 0), stop=(k == 1))
                nc.scalar.activation(t2[:, h: h + 1], t2ps[:], AF.Sigmoid,
                                     bias=l2bT[:, h: h + 1])

            ones = const.tile([P, 1], F32, name="ones_col")
            nc.vector.memset(ones[:], 1.0)
            p3 = work.tile([P, 6], F32, name="p3")
            nc.vector.tensor_mul(p3[:], t2[:], l3wT[:])
            ops_ = psp.tile([1, 6], F32, name="outps", tag="small", bufs=1)
            nc.tensor.matmul(ops_[:], ones[:], p3[:], start=True, stop=True)
            osb = work.tile([1, 6], F32, name="osb")
            nc.vector.tensor_add(osb[:], ops_[:], l3b[:])
            nc.sync.dma_start(out_d[:], osb[:])

    nc.compile()
    return nc


def _prep_inputs(inputs):
    x = np.asarray(inputs["x"], dtype=np.float32)
    adj = np.asarray(inputs["adj"])
    common = {
        "xT": np.ascontiguousarray(x.T.astype(NP_GAT)),
        "adjT": np.ascontiguousarray((adj.T > 0).astype(np.float16)),
        "ident": np.eye(P, dtype=np.float32),
    }
    for l, (Fin, F, g) in enumerate(LAYERS):
        a = np.asarray(inputs[f"a{l+1}"], dtype=np.float32)
        common[f"W{l+1}"] = np.ascontiguousarray(
            np.asarray(inputs[f"W{l+1}"], dtype=np.float32).astype(NP_GAT))
        common[f"aF{l+1}"] = np.ascontiguousarray(
            np.broadcast_to(a[:F], (P, F)).astype(NP_GAT))
        bv = np.asarray(inputs[f"b{l+1}"], dtype=np.float32)
        common[f"b{l+1}"] = np.ascontiguousarray(np.broadcast_to(bv, (P, F)))
        btc = np.zeros((P, 1), dtype=np.float32)
        btc[:F, 0] = bv
        common[f"bT{l+1}"] = btc
        aS = a[F:]
        i = np.arange(N)
        wsel = np.zeros((N, g), dtype=np.float32)
        wsel[i, i % g] = aS[i // g]
        common[f"wsel{l+1}"] = wsel.astype(NP_GAT)

    l2w = np.asarray(inputs["l2w"], dtype=np.float32)  # [6, 128, 256]
    common["L2T"] = np.ascontiguousarray(
        l2w.transpose(2, 0, 1).reshape(D2, 6 * P).astype(np.float16))
    common["l2bT"] = np.ascontiguousarray(
        np.asarray(inputs["l2b"], dtype=np.float32).T)       # [128, 6]
    common["l3wT"] = np.ascontiguousarray(
        np.asarray(inputs["l3w"], dtype=np.float32)[:, 0, :].T)  # [128, 6]
    common["l3b"] = np.ascontiguousarray(
        np.asarray(inputs["l3b"], dtype=np.float32).reshape(1, 6))

    l1w_flat = np.asarray(inputs["l1w"], dtype=np.float32).reshape(1536, 65536)
    l1b_flat = np.asarray(inputs["l1b"], dtype=np.float32).reshape(1536)
    common["l1bn"] = np.ascontiguousarray(l1b_flat.reshape(12, P))
    in_maps = []
    for c in range(NCORES):
        rows = l1w_flat[c * RSHARD:(c + 1) * RSHARD]       # [192, 65536]
        A = np.ascontiguousarray(
            rows.T.reshape(KCH, P, RSHARD).transpose(1, 0, 2).astype(np.float16))
        m = dict(common)
        m["A"] = A
        m["l1bs"] = np.ascontiguousarray(
            l1b_flat[c * RSHARD:(c + 1) * RSHARD].reshape(1, RSHARD))
        in_maps.append(m)
    return in_maps


def _ensure_ntff_hook():
    """Register the axon NTFF profile hook (the image's antenv lacks
    axon_hooks; supply it in sys.modules so bass_utils can trace)."""
    try:
        import types

        import antenv
        if "antenv.axon_hooks" not in sys.modules:
            mod = types.ModuleType("antenv.axon_hooks")
            mod._hook = None

            def _set(h, _m=mod):
                _m._hook = h

            def _get(_m=mod):
                return _m._hook

            mod.set_axon_ntff_profile_hook = _set
            mod.get_axon_ntff_profile_hook = _get
            sys.modules["antenv.axon_hooks"] = mod
            antenv.axon_hooks = mod
        from antenv.axon_hooks import (get_axon_ntff_profile_hook,
                                       set_axon_ntff_profile_hook)
        if get_axon_ntff_profile_hook() is None:
            from trn_agent_boot.trn_boot import _ntff_profile_via_ctypes
            set_axon_ntff_profile_hook(
                _ntff_profile_via_ctypes("/opt/axon/libaxon_pjrt.so"))
        return True
    except Exception as e:  # pragma: no cover - profiling is best-effort
        print(f"ntff hook unavailable: {e}", file=sys.stderr)
        return False


def kernel(**inputs) -> np.ndarray:
    if "nc" not in _CACHE:
        _CACHE["nc"] = _build()
    nc = _CACHE["nc"]
    in_maps = _prep_inputs(inputs)
    trace = bool(int(os.environ.get("BASS_KERNEL_TRACE", "0")))
    if trace:
        trace = _ensure_ntff_hook()
    res = run_bass_kernel_spmd(nc, in_maps, list(range(NCORES)), trace=trace)
    _CACHE["last_results"] = res
    return np.asarray(res.results[0]["out"]).reshape(6, 1)



# revision 11
# speedup vs baseline: 1.2437x; 1.2437x over previous
"""Trainium2 Bass kernel for nn_GAT_12232066859439.

3-layer GAT + 6-head MLP readout. Strategy (v2):
  - GAT replicated on all 8 cores. The scrambled-view attention collapses to
    att[i,j] = adj[i,j]*c[j, i//F] / rowsum, with c = exp(lrelu(e1[j] +
    e2T[j%F, u])). We scale haug rows by c (per-u, packed DVE ops) instead of
    materializing adj*e (the old strided-broadcast path), and fold e1 into the
    projection matmul via the precomposed column W @ a[:F].
  - The 100MB l1 matvec is sharded 192 rows/core, stored fp8e4 (x32) with
    fp8 xf (x256) and DoubleRow perf mode: half the HBM bytes and half the
    PE row time of fp16. adj is fp8 as well (exact 0/1).
  - Each core computes its partial l2 head contraction before the collective;
    a single AllReduce of [1,768] replaces the old AllGather + on-core l2.
  - All constants ride in a few packed DMAs (no small DIRECT2D storm).
"""
import os
import sys

sys.path.insert(0, "/opt/trn_rl_repo")

import numpy as np

import concourse.bacc as bacc
import concourse.bass as bass
import concourse.tile as tile
from concourse import mybir
from concourse.bass_utils import run_bass_kernel_spmd

F32 = mybir.dt.float32
F16 = mybir.dt.float16
F8 = mybir.dt.float8e4
AF = mybir.ActivationFunctionType
ALU = mybir.AluOpType
PM = mybir.MatmulPerfMode

P = 128
N = 1024
NCORES = 8
NCH = N // P
LAYERS = [(512, 128, 8), (128, 64, 16), (64, 64, 16)]
RSHARD = 1536 // NCORES      # 192 l1 rows per core
KCH = 65536 // P             # 512 k-tiles for the matvec
SA = 32.0                    # A fp8 scale
SX = 256.0                   # xf fp8 scale
WARM_COLLECTIVE = bool(int(os.environ.get("GAT_WARM_COLL", "1")))

# C16 packed fp16 const layout (columns)
C16_WAUG1 = 0                # [P, 4*129]
C16_WAUG2 = 516              # [P, 65]
C16_WAUG3 = 581              # [64, 65]
C16_WSEL1 = 646              # [P, 8*8]
C16_WSEL2 = 710              # [P, 8*16]
C16_WSEL3 = 838              # [P, 8*16]
C16_IDT = 966                # [P, 128] fp16 identity
C16_XT = 1094                # [P, 4*1024]
C16_COLS = 1094 + 4096

# C32 packed fp32 const layout
C32_IDENT = 0                # [P, 128]
C32_B1 = 128                 # [P, 128]
C32_B2 = 256                 # [64, 128]  (b2 twice)
C32_B3 = 384                 # [64, 128]  (b3 twice)
C32_COLS = 512

_CACHE = {}


def ts(i, n):
    return slice(i * n, (i + 1) * n)


def _build():
    nc = bacc.Bacc("TRN2", target_bir_lowering=False, debug=False,
                   num_devices=NCORES)

    c16_d = nc.dram_tensor("C16", [P, C16_COLS], F16, kind="ExternalInput")
    adj8_d = nc.dram_tensor("ADJ8", [P, NCH * N], F8, kind="ExternalInput")
    c32_d = nc.dram_tensor("C32", [P, C32_COLS], F32, kind="ExternalInput")
    c32r_d = nc.dram_tensor("C32R", [1, 1542], F32, kind="ExternalInput")
    a8_d = nc.dram_tensor("A8", [P, KCH, RSHARD], F8, kind="ExternalInput")
    l2s_d = nc.dram_tensor("L2S", [P, 2 * 768], F16, kind="ExternalInput")
    lb_d = nc.dram_tensor("LB", [P, 2], F32, kind="ExternalInput")
    out_d = nc.dram_tensor("out", [1, 6], F32, kind="ExternalOutput")

    with tile.TileContext(nc) as tc:
        with tc.tile_pool(name="const", bufs=1) as const, \
             tc.tile_pool(name="work", bufs=1) as work, \
             tc.tile_pool(name="dram", bufs=1, space="DRAM") as dram:

            # ---- optional collective warm-up (cold-start ~11.5us otherwise)
            if WARM_COLLECTIVE:
                wsb = const.tile([1, 8], F32, name="wsb")
                nc.vector.memset(wsb[:], 1.0)
                w_in = dram.tile([1, 8], F32, name="w_in")
                w_out = dram.tile([1, 8], F32, name="w_out",
                                  addr_space="Shared")
                nc.sync.dma_start(w_in[:], wsb[:])
                nc.gpsimd.collective_compute(
                    "AllReduce", ALU.add,
                    replica_groups=[list(range(NCORES))],
                    ins=[w_in.opt()], outs=[w_out.opt()])

            # ---- packed constant loads (order = need order) ----
            c16 = const.tile([P, C16_COLS], F16, name="c16")
            nc.sync.dma_start(c16[:, 0:C16_XT], c16_d[:, 0:C16_XT])
            for i in range(4):
                s = slice(C16_XT + i * N, C16_XT + (i + 1) * N)
                nc.sync.dma_start(c16[:, s], c16_d[:, s])
            c32 = const.tile([P, C32_COLS], F32, name="c32")
            nc.sync.dma_start(c32[:], c32_d[:])
            c32r = const.tile([1, 1542], F32, name="c32r")
            nc.sync.dma_start(c32r[:], c32r_d[:])
            l2s = const.tile([P, 2 * 768], F16, name="l2s")
            nc.sync.dma_start(l2s[:], l2s_d[:])
            lb = const.tile([P, 2], F32, name="lb")
            nc.sync.dma_start(lb[:], lb_d[:])
            adj8 = const.tile([P, NCH * N], F8, name="adj8")
            for i in range(4):
                nc.sync.dma_start(adj8[:, ts(i, 2 * N)],
                                  adj8_d[:, ts(i, 2 * N)])
            a8 = const.tile([P, KCH, RSHARD], F8, name="a8")
            NSLAB = 32
            KSL = KCH // NSLAB
            for s in range(NSLAB):
                nc.sync.dma_start(a8[:, ts(s, KSL), :], a8_d[:, ts(s, KSL), :])

            ident = c32[:, C32_IDENT:C32_IDENT + 128]
            idt = c16[:, C16_IDT:C16_IDT + 128]
            waug = [c16[:, C16_WAUG1:C16_WAUG1 + 516],
                    c16[:, C16_WAUG2:C16_WAUG2 + 65],
                    c16[0:64, C16_WAUG3:C16_WAUG3 + 65]]
            wsel = [c16[:, C16_WSEL1:C16_WSEL1 + 64],
                    c16[:, C16_WSEL2:C16_WSEL2 + 128],
                    c16[:, C16_WSEL3:C16_WSEL3 + 128]]
            bb = [c32[:, C32_B1:C32_B1 + 128],
                  c32[0:64, C32_B2:C32_B2 + 128],
                  c32[0:64, C32_B3:C32_B3 + 128]]
            xT = c16[:, C16_XT:C16_XT + 4 * N]

            xf3 = work.tile([P, KCH], F8, name="xf3")
            hT = None  # transposed h for next layer [F, N]

            gp = const.tile([1, 8], F32, name="gp")

            # ---- GAT layers ----
            with tc.tile_pool(name="psg", bufs=1, space="PSUM") as psg:
                for l, (Fin, F, g) in enumerate(LAYERS):
                    nk = max(1, Fin // P)
                    kp = min(P, Fin)
                    FA = F + 1  # h features + scaled-ones column

                    haug = work.tile([P, FA * NCH], F16, name=f"haug{l}")
                    haugv = haug.rearrange("p (f k) -> p f k", k=NCH)
                    e1 = work.tile([P, NCH], F32, name=f"e1_{l}")
                    nc.vector.memset(haug[:, F * NCH:FA * NCH], 1.0)

                    for kc in range(NCH):
                        hp = psg.tile([P, 129], F32, name=f"hp{l}_{kc}",
                                      tag="hp", bufs=2)
                        for ks in range(nk):
                            if l == 0:
                                lhsT = xT[:, ks * N + kc * P:
                                          ks * N + (kc + 1) * P]
                                rhs = waug[0][:, ts(ks, 129)]
                            else:
                                lhsT = hT[0:kp, ts(kc, P)]
                                rhs = waug[l]
                            nc.tensor.matmul(hp[:, 0:FA], lhsT, rhs,
                                             start=(ks == 0),
                                             stop=(ks == nk - 1))
                        nc.scalar.activation(haugv[:, 0:F, kc], hp[:, 0:F],
                                             AF.Copy)
                        nc.scalar.activation(e1[:, kc:kc + 1], hp[:, F:F + 1],
                                             AF.Copy)

                    # e2T[v, u] (v = j mod F); for F=64 duplicate upper half
                    e2p = psg.tile([P, 16], F32, name=f"e2p{l}", tag="e2p",
                                   bufs=1)
                    for m in range(NCH):
                        nc.tensor.matmul(e2p[0:F, 0:g], haugv[:, 0:F, m],
                                         wsel[l][:, ts(m, g)],
                                         start=(m == 0), stop=(m == NCH - 1))
                    e2T = work.tile([P, g], F32, name=f"e2T{l}")
                    nc.vector.tensor_copy(e2T[0:F, :], e2p[0:F, 0:g])
                    if F == 64:
                        nc.sync.dma_start(e2T[64:128, :], e2T[0:64, :])

                    # c[j, u] = exp(lrelu(e1[j] + e2T[j%F, u])), u-major
                    s_scr = work.tile([P, g * NCH], F32, name=f"sscr{l}")
                    sv = s_scr.rearrange("p (u k) -> p u k", k=NCH)
                    nc.vector.tensor_add(
                        sv, e1[:].broadcast_to([P, NCH, g]).rearrange(
                            "p k u -> p u k"),
                        e2T[:].broadcast_to([P, g, NCH]))
                    nc.vector.scalar_tensor_tensor(s_scr[:], s_scr[:], 0.2,
                                                   s_scr[:], ALU.mult,
                                                   ALU.max)
                    e_all = work.tile([P, g * NCH], F16, name=f"eall{l}")
                    nc.scalar.activation(e_all[:], s_scr[:], AF.Exp)

                    nc.gpsimd.memset(gp[:], float(l))

                    if l == 0:
                        hT = work.tile([P, N], F16, name="hT0")
                        for u in range(g):
                            M2 = work.tile([P, 1032], F16, name=f"M2_{l}_{u}",
                                           tag="M2", bufs=4)
                            M2v = M2.rearrange("p (f k) -> p f k", k=NCH)
                            nc.vector.tensor_mul(
                                M2v[:, 0:FA, :], haugv,
                                e_all[:, ts(u, NCH)].broadcast_to(
                                    [P, NCH, FA]).rearrange("p k f -> p f k"))
                            nps = psg.tile([P, 130], F32, name=f"nps{l}_{u}",
                                           tag="nps", bufs=2)
                            for kc in range(NCH):
                                nc.tensor.matmul(
                                    nps[:, 0:FA],
                                    adj8[:, kc * N + u * P:
                                         kc * N + (u + 1) * P],
                                    M2v[:, 0:FA, kc],
                                    start=(kc == 0), stop=(kc == NCH - 1))
                            rd = work.tile([P, 2], F32, name=f"rd{l}_{u}",
                                           tag="rd", bufs=3)
                            nc.vector.reciprocal(rd[:, 0:1], nps[:, F:F + 1])
                            y = work.tile([P, P], F32, name=f"y{l}_{u}",
                                          tag="y", bufs=3)
                            nc.scalar.activation(y[:], nps[:, 0:F], AF.Relu,
                                                 scale=rd[:, 0:1])
                            hn = work.tile([P, P], F16, name=f"hn{l}_{u}",
                                           tag="hn", bufs=3)
                            nc.vector.tensor_add(hn[:], y[:], bb[0])
                            tp = psg.tile([P, P], F16, name=f"tp{l}_{u}",
                                          tag="tp", bufs=2)
                            nc.tensor.transpose(tp[:], hn[:], idt)
                            nc.scalar.activation(hT[:, ts(u, P)], tp[:],
                                                 AF.Relu)
                    else:
                        if l == 1:
                            hTn = work.tile([P, N], F16, name="hT1")
                        for m in range(NCH):
                            nps = psg.tile([P, 130], F32, name=f"nps{l}_{m}",
                                           tag="nps", bufs=2)
                            for hh in range(2):
                                u = 2 * m + hh
                                M2 = work.tile([P, 1032], F16,
                                               name=f"M2_{l}_{m}_{hh}",
                                               tag="M2", bufs=4)
                                M2v = M2.rearrange("p (f k) -> p f k", k=NCH)
                                nc.vector.tensor_mul(
                                    M2v[:, 0:FA, :], haugv,
                                    e_all[:, ts(u, NCH)].broadcast_to(
                                        [P, NCH, FA]).rearrange(
                                            "p k f -> p f k"))
                                for kc in range(NCH):
                                    nc.tensor.matmul(
                                        nps[0:64, ts(hh, FA)],
                                        adj8[:, kc * N + u * 64:
                                             kc * N + (u + 1) * 64],
                                        M2v[:, 0:FA, kc],
                                        start=(kc == 0), stop=(kc == NCH - 1))
                            rd2 = work.tile([P, 2], F32, name=f"rd{l}_{m}",
                                            tag="rd", bufs=3)
                            nc.vector.reciprocal(
                                rd2[0:64, :],
                                nps.rearrange("p (h f) -> p h f",
                                              f=FA)[0:64, :, F])
                            y2 = work.tile([P, P], F32, name=f"y{l}_{m}",
                                           tag="y", bufs=3)
                            for hh in range(2):
                                nc.scalar.activation(
                                    y2[0:64, ts(hh, F)],
                                    nps[0:64, hh * FA: hh * FA + F],
                                    AF.Relu, scale=rd2[0:64, hh:hh + 1])
                            hn2 = work.tile([P, P], F16, name=f"hn{l}_{m}",
                                            tag="hn", bufs=3)
                            nc.vector.tensor_add(hn2[0:64, :], y2[0:64, :],
                                                 bb[l])
                            tp2 = psg.tile([P, P], F16, name=f"tp{l}_{m}",
                                           tag="tp", bufs=2)
                            nc.tensor.transpose(tp2[:, 0:64], hn2[0:64, :],
                                                idt[0:64, 0:64])
                            if l == 1:
                                nc.scalar.activation(
                                    hTn[0:64, m * P:m * P + 64],
                                    tp2[0:64, 0:64], AF.Relu)
                                nc.scalar.activation(
                                    hTn[0:64, m * P + 64:(m + 1) * P],
                                    tp2[64:128, 0:64], AF.Relu)
                            else:
                                nc.scalar.activation(xf3[:, ts(m, 64)],
                                                     tp2[:, 0:64], AF.Relu,
                                                     scale=SX)
                        if l == 1:
                            hT = hTn

            with tc.tile_pool(name="pst", bufs=1, space="PSUM") as pst:
                # ---- l1 matvec: t1 = A8.T @ xf (fp8 DoubleRow) ----
                # Diagonal-pair trick: step t consumes k-tiles
                # {2t, 2t+1, 256+2t, 256+2t+1} as (s, m) with s-stride 256
                # (DoubleRow weights need Ko step % 16 == 0). Only diagonal
                # blocks m==j of out[m, j*192+r] are kept, so
                # t1 = out[0, 0:192] + out[1, 192:384].
                t1ps = pst.tile([2, 2 * RSHARD], F32, name="t1ps")
                xfv = xf3.rearrange("p (s tm) -> p s tm", s=2)
                a8v = a8.rearrange("p (t s j) r -> p t s (j r)", s=2, j=2)
                NP4 = KCH // 4
                for t in range(NP4):
                    nc.tensor.matmul(t1ps[:], xfv[:, :, 2 * t:2 * t + 2],
                                     a8v[:, t, :, :],
                                     start=(t == 0), stop=(t == NP4 - 1),
                                     perf_mode=PM.DoubleRow)
                t1c = work.tile([2, 2 * RSHARD], F32, name="t1c")
                nc.vector.tensor_copy(t1c[:], t1ps[:])
                t1b = work.tile([1, RSHARD], F32, name="t1b")
                nc.sync.dma_start(t1b[:], t1c[1:2, RSHARD:2 * RSHARD])
                t1row = work.tile([1, RSHARD], F32, name="t1row")
                nc.vector.tensor_add(t1row[:], t1c[0:1, 0:RSHARD], t1b[:])

                # shard onto partitions; t1 = relu(z/(SA*SX) + b)
                ttpa = pst.tile([P, 1], F32, name="ttpa")
                nc.tensor.transpose(ttpa[:], t1row[:, 0:P], ident[0:1, 0:1])
                ttpb = pst.tile([64, 1], F32, name="ttpb")
                nc.tensor.transpose(ttpb[:], t1row[:, P:RSHARD],
                                    ident[0:1, 0:1])
                t1sa = work.tile([P, 1], F16, name="t1sa")
                nc.scalar.activation(t1sa[:], ttpa[:], AF.Relu,
                                     scale=1.0 / (SA * SX), bias=lb[:, 0:1])
                t1sb = work.tile([64, 1], F16, name="t1sb")
                nc.scalar.activation(t1sb[:], ttpb[:], AF.Relu,
                                     scale=1.0 / (SA * SX), bias=lb[0:64, 1:2])

                # partial l2: z2[h*128+o] = sum_{r in shard} l2w[h,o,r]*t1[r]
                z2 = work.tile([1, 768], F32, name="z2")
                for half in range(2):
                    ps2 = pst.tile([1, 384], F32, name=f"ps2_{half}",
                                   tag="ps2", bufs=2)
                    nc.tensor.matmul(ps2[:], t1sa[:],
                                     l2s[:, half * 384:half * 384 + 384],
                                     start=True, stop=False)
                    nc.tensor.matmul(
                        ps2[:], t1sb[:],
                        l2s[0:64, 768 + half * 384:768 + half * 384 + 384],
                        start=False, stop=True)
                    nc.vector.tensor_copy(z2[:, ts(half, 384)], ps2[:])

                rr_in = dram.tile([1, 768], F32, name="rr_in")
                rr_out = dram.tile([1, 768], F32, name="rr_out",
                                   addr_space="Shared")
                nc.sync.dma_start(rr_in[:], z2[:])
                nc.gpsimd.collective_compute(
                    "AllReduce", ALU.add,
                    replica_groups=[list(range(NCORES))],
                    ins=[rr_in.opt()], outs=[rr_out.opt()])

                # ---- tail: sigmoid(z2 + l2b), out = l3w . t2 + l3b ----
                zz = work.tile([1, 768], F32, name="zz")
                nc.sync.dma_start(zz[:], rr_out[:])
                nc.vector.tensor_add(zz[:], zz[:], c32r[:, 0:768])
                t2 = work.tile([1, 768], F32, name="t2")
                nc.scalar.activation(t2[:], zz[:], AF.Sigmoid)
                p3 = work.tile([1, 768], F32, name="p3")
                nc.vector.tensor_mul(p3[:], t2[:], c32r[:, 768:1536])
                o6 = work.tile([1, 6], F32, name="o6")
                nc.vector.reduce_sum(o6[:],
                                     p3.rearrange("p (h o) -> p h o", o=P),
                                     axis=mybir.AxisListType.X)
                oo = work.tile([1, 6], F32, name="oo")
                nc.vector.tensor_add(oo[:], o6[:], c32r[:, 1536:1542])
                nc.sync.dma_start(out_d[:], oo[:])

    nc.compile()
    return nc


def _prep_inputs(inputs):
    f8 = mybir.dt.np(F8)
    x = np.asarray(inputs["x"], dtype=np.float32)
    adj = np.asarray(inputs["adj"])

    def chunked(arr, nch):
        # [nch*P, C] -> [P, nch*C] with block kc at cols [kc*C:(kc+1)*C]
        c = arr.shape[1]
        return arr.reshape(nch, P, c).transpose(1, 0, 2).reshape(P, nch * c)

    c16 = np.zeros((P, C16_COLS), dtype=np.float16)
    c32 = np.zeros((P, C32_COLS), dtype=np.float32)
    for l, (Fin, F, g) in enumerate(LAYERS):
        W = np.asarray(inputs[f"W{l+1}"], dtype=np.float64)
        a = np.asarray(inputs[f"a{l+1}"], dtype=np.float64)
        b = np.asarray(inputs[f"b{l+1}"], dtype=np.float32)
        waug = np.concatenate([W, (W @ a[:F])[:, None]], axis=1)  # [Fin,F+1]
        off = [C16_WAUG1, C16_WAUG2, C16_WAUG3][l]
        if l == 0:
            c16[:, off:off + 516] = chunked(waug, 4).astype(np.float16)
        else:
            c16[0:Fin, off:off + F + 1] = waug.astype(np.float16)
        aS = a[F:]
        i = np.arange(N)
        wm = np.zeros((N, g), dtype=np.float64)
        wm[i, i % g] = aS[i // g]
        woff = [C16_WSEL1, C16_WSEL2, C16_WSEL3][l]
        c16[:, woff:woff + NCH * g] = chunked(wm, NCH).astype(np.float16)
        boff = [C32_B1, C32_B2, C32_B3][l]
        if l == 0:
            c32[:, boff:boff + 128] = np.broadcast_to(b, (P, F))
        else:
            c32[0:64, boff:boff + 128] = np.broadcast_to(
                np.concatenate([b, b]), (64, 2 * F))
    c16[:, C16_IDT:C16_IDT + 128] = np.eye(P, dtype=np.float16)
    c16[:, C16_XT:] = chunked(np.ascontiguousarray(x.T), 4).astype(np.float16)
    c32[:, C32_IDENT:C32_IDENT + 128] = np.eye(P, dtype=np.float32)

    adjT = (adj.T > 0).astype(np.float32)
    adj8 = chunked(adjT, NCH).astype(f8)

    l2w = np.asarray(inputs["l2w"], dtype=np.float32)   # [6,128,256]
    l2b = np.asarray(inputs["l2b"], dtype=np.float32)
    l3w = np.asarray(inputs["l3w"], dtype=np.float32)   # [6,1,128]
    l3b = np.asarray(inputs["l3b"], dtype=np.float32)
    c32r = np.zeros((1, 1542), dtype=np.float32)
    c32r[0, 0:768] = l2b.reshape(-1)
    c32r[0, 768:1536] = l3w[:, 0, :].reshape(-1)
    c32r[0, 1536:1542] = l3b.reshape(-1)

    l1w_flat = np.asarray(inputs["l1w"], dtype=np.float32).reshape(1536, 65536)
    l1b_flat = np.asarray(inputs["l1b"], dtype=np.float32).reshape(1536)
    l1w_q = (l1w_flat * SA).astype(f8)

    # t1 index r = h*256 + t contracts only into head h: block-diagonal
    l2big = np.zeros((1536, 768), dtype=np.float32)
    for h in range(6):
        l2big[ts(h, 256), ts(h, 128)] = l2w[h].T        # [256,128]

    common = dict(C16=c16, ADJ8=adj8, C32=c32, C32R=c32r)
    in_maps = []
    for c in range(NCORES):
        rows = l1w_q[ts(c, RSHARD)]                     # [192, 65536]
        # k(p, t) = (m*128 + (p//64)*64 + t%64)*64 + p%64 ;  t = m*64 + il
        A = rows.reshape(RSHARD, 8, 2, 64, 64)          # [r, m, par, il, f]
        A = A.transpose(2, 4, 1, 3, 0).reshape(P, KCH, RSHARD)
        # DoubleRow block order: step t holds tiles [2t, 2t+1, 256+2t,
        # 256+2t+1] adjacently (s-stride 384 fits the 16-bit ISA step)
        tt = np.arange(KCH // 4)
        perm = np.stack([2 * tt, 2 * tt + 1, 256 + 2 * tt, 257 + 2 * tt],
                        axis=1).reshape(-1)
        A = A[:, perm, :]
        sub = l2big[ts(c, RSHARD)]                      # [192, 768]
        l2sa = sub[0:128]
        l2sb = np.zeros((P, 768), dtype=np.float32)
        l2sb[0:64] = sub[128:192]
        lbv = np.zeros((P, 2), dtype=np.float32)
        lbv[:, 0] = l1b_flat[c * RSHARD:c * RSHARD + 128]
        lbv[0:64, 1] = l1b_flat[c * RSHARD + 128:(c + 1) * RSHARD]
        m = dict(common)
        m["A8"] = np.ascontiguousarray(A)
        m["L2S"] = np.concatenate([l2sa, l2sb], axis=1).astype(np.float16)
        m["LB"] = lbv
        in_maps.append(m)
    return in_maps


def _ensure_ntff_hook():
    """Register the axon NTFF profile hook (the image's antenv lacks
    axon_hooks; supply it in sys.modules so bass_utils can trace)."""
    try:
        import types

        import antenv
        if "antenv.axon_hooks" not in sys.modules:
            mod = types.ModuleType("antenv.axon_hooks")
            mod._hook = None

            def _set(h, _m=mod):
                _m._hook = h

            def _get(_m=mod):
                return _m._hook

            mod.set_axon_ntff_profile_hook = _set
            mod.get_axon_ntff_profile_hook = _get
            sys.modules["antenv.axon_hooks"] = mod
            antenv.axon_hooks = mod
        from antenv.axon_hooks import (get_axon_ntff_profile_hook,
                                       set_axon_ntff_profile_hook)
        if get_axon_ntff_profile_hook() is None:
            from trn_agent_boot.trn_boot import _ntff_profile_via_ctypes
            set_axon_ntff_profile_hook(
                _ntff_profile_via_ctypes("/opt/axon/libaxon_pjrt.so"))
        return True
    except Exception as e:  # pragma: no cover - profiling is best-effort
        print(f"ntff hook unavailable: {e}", file=sys.stderr)
        return False


def kernel(**inputs) -> np.ndarray:
    if "nc" not in _CACHE:
        _CACHE["nc"] = _build()
    nc = _CACHE["nc"]
    in_maps = _prep_inputs(inputs)
    trace = bool(int(os.environ.get("BASS_KERNEL_TRACE", "0")))
    if trace:
        trace = _ensure_ntff_hook()
    res = run_bass_kernel_spmd(nc, in_maps, list(range(NCORES)), trace=trace)
    _CACHE["last_results"] = res
    return np.asarray(res.results[0]["out"],
                      dtype=np.float32).reshape(6, 1)


# revision 13
# speedup vs baseline: 1.4122x; 1.1355x over previous
"""Trainium2 Bass kernel for nn_GAT_12232066859439.

3-layer GAT + 6-head MLP readout. Strategy (v3):
  - GAT replicated on all 8 cores. The scrambled-view attention collapses to
    att[i,j] = adj[i,j]*c[j, i//F] / rowsum with c = exp(lrelu(e1[j] +
    e2T[j%F, u])); e1 is folded into the projection matmul via the
    precomposed column W @ a[:F].
    Layer 1 (F=128): scale haug per-u with tensor_scalar (c is constant per
    i-block), one matmul per (u, kc).
    Layers 2/3 (F=64): materialize w = adj*c on the lhs side (one DVE op per
    m-chunk), one matmul per (m, kc).
  - The 100MB l1 matvec is sharded 192 rows/core, stored fp8e4 (x32) with
    fp8 xf (x256), DoubleRow perf mode, diagonal-pair packing. Layer-3
    output writes straight into xf (no transpose) and the matvec steps are
    interleaved into the layer-3 loop so the PE never idles.
  - Each core computes its partial l2 head contraction; one AllReduce of
    [1,768] replaces gather + on-core l2. Tail runs as [6,128].
"""
import os
import sys

sys.path.insert(0, "/opt/trn_rl_repo")

import numpy as np

import concourse.bacc as bacc
import concourse.bass as bass
import concourse.tile as tile
from concourse import mybir
from concourse.bass_utils import run_bass_kernel_spmd

F32 = mybir.dt.float32
F16 = mybir.dt.float16
F8 = mybir.dt.float8e4
AF = mybir.ActivationFunctionType
ALU = mybir.AluOpType
PM = mybir.MatmulPerfMode

P = 128
N = 1024
NCORES = 8
NCH = N // P
LAYERS = [(512, 128, 8), (128, 64, 16), (64, 64, 16)]
RSHARD = 1536 // NCORES      # 192 l1 rows per core
KCH = 65536 // P             # 512 k-tiles for the matvec
SA = 32.0                    # A fp8 scale
SX = 256.0                   # xf fp8 scale
WARM_COLLECTIVE = bool(int(os.environ.get("GAT_WARM_COLL", "0")))

# C16 packed fp16 const layout (columns)
C16_WAUG1 = 0                # [P, 4*129]
C16_WAUG2 = 516              # [P, 65]
C16_WAUG3 = 581              # [64, 65]
C16_WSEL1 = 646              # [P, 8*8]
C16_WSEL2 = 710              # [P, 8*16]
C16_WSEL3 = 838              # [P, 8*16]
C16_IDT = 966                # [P, 128] fp16 identity
C16_XT = 1094                # [P, 4*1024]
C16_COLS = 1094 + 4096

# C32 packed fp32 const layout
C32_IDENT = 0                # [P, 128]
C32_B1 = 128                 # [P, 128]
C32_B2 = 256                 # [P, 64]
C32_B3 = 320                 # [P, 64]
C32_COLS = 384

_CACHE = {}


def ts(i, n):
    return slice(i * n, (i + 1) * n)


def _build():
    nc = bacc.Bacc("TRN2", target_bir_lowering=False, debug=False,
                   num_devices=NCORES)

    c16_d = nc.dram_tensor("C16", [P, C16_COLS], F16, kind="ExternalInput")
    adj8_d = nc.dram_tensor("ADJ8", [P, NCH * N], F8, kind="ExternalInput")
    c32_d = nc.dram_tensor("C32", [P, C32_COLS], F32, kind="ExternalInput")
    c32r_d = nc.dram_tensor("C32R", [6, 257], F32, kind="ExternalInput")
    a8_d = nc.dram_tensor("A8", [P, KCH, RSHARD], F8, kind="ExternalInput")
    l2s_d = nc.dram_tensor("L2S", [P, 2 * 768], F16, kind="ExternalInput")
    lb_d = nc.dram_tensor("LB", [P, 2], F32, kind="ExternalInput")
    out_d = nc.dram_tensor("out", [6, 1], F32, kind="ExternalOutput")

    with tile.TileContext(nc) as tc:
        with tc.tile_pool(name="const", bufs=1) as const, \
             tc.tile_pool(name="work", bufs=1) as work, \
             tc.tile_pool(name="pmv", bufs=1, space="PSUM") as pmv, \
             tc.tile_pool(name="dram", bufs=1, space="DRAM") as dram:

            if WARM_COLLECTIVE:
                wsb = const.tile([1, 8], F32, name="wsb")
                nc.vector.memset(wsb[:], 1.0)
                w_in = dram.tile([1, 8], F32, name="w_in")
                w_out = dram.tile([1, 8], F32, name="w_out",
                                  addr_space="Shared")
                nc.sync.dma_start(w_in[:], wsb[:])
                nc.gpsimd.collective_compute(
                    "AllReduce", ALU.add,
                    replica_groups=[list(range(NCORES))],
                    ins=[w_in.opt()], outs=[w_out.opt()])

            # ---- packed constant loads (each dma_start fans out over all
            # DMA queues; trigger issue on sync costs ~0.65us apiece) ----
            c16 = const.tile([P, C16_COLS], F16, name="c16")
            nc.sync.dma_start(c16[:], c16_d[:])
            c32 = const.tile([P, C32_COLS], F32, name="c32")
            nc.sync.dma_start(c32[:], c32_d[:])
            adj8 = const.tile([P, NCH * N], F8, name="adj8")
            for i in range(2):
                nc.sync.dma_start(adj8[:, ts(i, 4 * N)],
                                  adj8_d[:, ts(i, 4 * N)])
            c32r = const.tile([6, 257], F32, name="c32r")
            nc.sync.dma_start(c32r[:], c32r_d[:])
            l2s = const.tile([P, 2 * 768], F16, name="l2s")
            nc.sync.dma_start(l2s[:], l2s_d[:])
            lb = const.tile([P, 2], F32, name="lb")
            nc.sync.dma_start(lb[:], lb_d[:])
            a8 = const.tile([P, KCH, RSHARD], F8, name="a8")
            for s in range(4):
                nc.sync.dma_start(a8[:, ts(s, 128), :], a8_d[:, ts(s, 128), :])

            ident = c32[:, C32_IDENT:C32_IDENT + 128]
            idt = c16[:, C16_IDT:C16_IDT + 128]
            waug = [c16[:, C16_WAUG1:C16_WAUG1 + 516],
                    c16[:, C16_WAUG2:C16_WAUG2 + 65],
                    c16[0:64, C16_WAUG3:C16_WAUG3 + 65]]
            wsel = [c16[:, C16_WSEL1:C16_WSEL1 + 64],
                    c16[:, C16_WSEL2:C16_WSEL2 + 128],
                    c16[:, C16_WSEL3:C16_WSEL3 + 128]]
            bb = [c32[:, C32_B1:C32_B1 + 128],
                  c32[:, C32_B2:C32_B2 + 64],
                  c32[:, C32_B3:C32_B3 + 64]]
            xT = c16[:, C16_XT:C16_XT + 4 * N]
            adjv = adj8.rearrange("p (i k) -> p i k", k=NCH)  # adj8 i-major

            xf3 = work.tile([P, KCH], F8, name="xf3")
            xfv = xf3.rearrange("p (q s w) -> p q s w", s=2, w=16)
            a8v = a8.rearrange("p (v s j) r -> p v s (j r)", s=2, j=2)
            t1ps = pmv.tile([2, 2 * RSHARD], F32, name="t1ps")
            hT = None

            # ---- GAT layers ----
            with tc.tile_pool(name="psg", bufs=1, space="PSUM") as psg:
                for l, (Fin, F, g) in enumerate(LAYERS):
                    nk = max(1, Fin // P)
                    kp = min(P, Fin)
                    FA = F + 1  # h features + ones column (kc-major blocks)

                    haug = work.tile([P, FA * NCH], F16, name=f"haug{l}")
                    e1 = work.tile([P, NCH], F32, name=f"e1_{l}")
                    nc.vector.memset(
                        haug.rearrange("p (k f) -> p k f", f=FA)[:, :, F], 1.0)

                    for kc in range(NCH):
                        hp = psg.tile([P, 129], F32, name=f"hp{l}_{kc}",
                                      tag="hp", bufs=2)
                        for ks in range(nk):
                            if l == 0:
                                lhsT = xT[:, ks * N + kc * P:
                                          ks * N + (kc + 1) * P]
                                rhs = waug[0][:, ts(ks, 129)]
                            else:
                                lhsT = hT[0:kp, ts(kc, P)]
                                rhs = waug[l]
                            nc.tensor.matmul(hp[:, 0:FA], lhsT, rhs,
                                             start=(ks == 0),
                                             stop=(ks == nk - 1))
                        nc.scalar.activation(haug[:, kc * FA:kc * FA + F],
                                             hp[:, 0:F], AF.Copy)
                        nc.scalar.activation(e1[:, kc:kc + 1], hp[:, F:F + 1],
                                             AF.Copy)

                    # e2T[v, u] (v = j mod F); for F=64 duplicate upper half
                    e2p = psg.tile([P, 129], F32, name=f"e2p{l}", tag="hp",
                                   bufs=2)
                    for m in range(NCH):
                        nc.tensor.matmul(e2p[0:F, 0:g],
                                         haug[:, m * FA:m * FA + F],
                                         wsel[l][:, ts(m, g)],
                                         start=(m == 0), stop=(m == NCH - 1))
                    e2T = work.tile([P, g], F32, name=f"e2T{l}")
                    nc.vector.tensor_copy(e2T[0:F, :], e2p[0:F, 0:g])
                    if F == 64:
                        nc.sync.dma_start(e2T[64:128, :], e2T[0:64, :])

                    # c[j, u] = exp(lrelu(e1[j] + e2T[j%F, u])), u-major
                    s_scr = work.tile([P, g * NCH], F32, name=f"sscr{l}")
                    sv = s_scr.rearrange("p (u k) -> p u k", k=NCH)
                    nc.vector.tensor_add(
                        sv, e1[:].broadcast_to([P, NCH, g]).rearrange(
                            "p k u -> p u k"),
                        e2T[:].broadcast_to([P, g, NCH]))
                    nc.vector.scalar_tensor_tensor(s_scr[:], s_scr[:], 0.2,
                                                   s_scr[:], ALU.mult,
                                                   ALU.max)
                    e_all = work.tile([P, g * NCH], F32, name=f"eall{l}")
                    nc.scalar.activation(e_all[:], s_scr[:], AF.Exp)

                    if l == 0:
                        hT = work.tile([P, N], F16, name="hT0")
                        for u in range(g):
                            M2 = work.tile([P, FA * NCH], F16,
                                           name=f"M2_{l}_{u}", tag="M2",
                                           bufs=3)
                            nps = psg.tile([P, 129], F32, name=f"nps{l}_{u}",
                                           tag="nps", bufs=3)
                            for kc in range(NCH):
                                nc.vector.tensor_scalar_mul(
                                    M2[:, ts(kc, FA)], haug[:, ts(kc, FA)],
                                    e_all[:, u * NCH + kc:u * NCH + kc + 1])
                                nc.tensor.matmul(
                                    nps[:, 0:FA],
                                    adjv[:, ts(u, P), kc],
                                    M2[:, ts(kc, FA)],
                                    start=(kc == 0), stop=(kc == NCH - 1))
                            rd = work.tile([P, 1], F32, name=f"rd{l}_{u}",
                                           tag="rd", bufs=3)
                            nc.vector.reciprocal(rd[:], nps[:, F:F + 1])
                            y = work.tile([P, P], F32, name=f"y{l}_{u}",
                                          tag="y", bufs=3)
                            nc.scalar.activation(y[:], nps[:, 0:F], AF.Relu,
                                                 scale=rd[:])
                            hn = work.tile([P, P], F16, name=f"hn{l}_{u}",
                                           tag="hn", bufs=3)
                            nc.vector.tensor_add(hn[:], y[:], bb[0])
                            tp = psg.tile([P, P], F16, name=f"tp{l}_{u}",
                                          tag="tp", bufs=2)
                            nc.tensor.transpose(tp[:], hn[:], idt)
                            nc.scalar.activation(hT[:, ts(u, P)], tp[:],
                                                 AF.Relu)
                    else:
                        if l == 1:
                            hTn = work.tile([P, N], F16, name="hT1")
                        for m in range(NCH):
                            # w[p, kc*128+i] = adj[j, i] * c[j, u(i)]
                            w = work.tile([P, NCH * P], F16, name=f"w{l}_{m}",
                                          tag="M2", bufs=3)
                            nc.vector.tensor_mul(
                                w.rearrange("p (k h i) -> p k h i",
                                            h=2, i=64),
                                adjv[:, ts(m, P), :].rearrange(
                                    "p (h i) k -> p k h i", h=2),
                                e_all[:, ts(m, 16)].rearrange(
                                    "p (h k) -> p k h",
                                    k=NCH).broadcast_to([P, NCH, 2, 64]))
                            nps = psg.tile([P, 129], F32, name=f"nps{l}_{m}",
                                           tag="nps", bufs=3)
                            for kc in range(NCH):
                                nc.tensor.matmul(nps[:, 0:FA],
                                                 w[:, ts(kc, P)],
                                                 haug[:, ts(kc, FA)],
                                                 start=(kc == 0),
                                                 stop=(kc == NCH - 1))
                            rd = work.tile([P, 1], F32, name=f"rd{l}_{m}",
                                           tag="rd", bufs=3)
                            nc.vector.reciprocal(rd[:], nps[:, F:F + 1])
                            y = work.tile([P, P], F32, name=f"y{l}_{m}",
                                          tag="y", bufs=3)
                            nc.scalar.activation(y[:, 0:F], nps[:, 0:F],
                                                 AF.Relu, scale=rd[:])
                            hn = work.tile([P, P], F16, name=f"hn{l}_{m}",
                                           tag="hn", bufs=3)
                            nc.vector.tensor_add(hn[:, 0:F], y[:, 0:F],
                                                 bb[l])
                            if l == 1:
                                tp = psg.tile([P, P], F16, name=f"tp{l}_{m}",
                                              tag="tp", bufs=2)
                                nc.tensor.transpose(tp[0:64, :], hn[:, 0:64],
                                                    idt)
                                nc.scalar.activation(hTn[0:64, ts(m, P)],
                                                     tp[0:64, :], AF.Relu)
                            else:
                                nc.scalar.activation(xf3[:, ts(m, 64)],
                                                     hn[:, 0:64], AF.Relu,
                                                     scale=SX)
                                # matvec steps for this chunk: step V eats
                                # xf cols {c0, c0+1, c0+16, c0+17},
                                # c0 = 32*(V//8) + 2*(V%8)
                                for V in range(m * 16, m * 16 + 16):
                                    nc.tensor.matmul(
                                        t1ps[:],
                                        xfv[:, V // 8, :,
                                            2 * (V % 8):2 * (V % 8) + 2],
                                        a8v[:, V, :, :],
                                        start=(V == 0), stop=(V == 127),
                                        perf_mode=PM.DoubleRow)
                        if l == 1:
                            hT = hTn

            with tc.tile_pool(name="pst", bufs=1, space="PSUM") as pst:
                # t1 = relu(z/(SA*SX) + b) on partitions, then partial l2
                t1c = work.tile([2, 2 * RSHARD], F32, name="t1c")
                nc.vector.tensor_copy(t1c[:], t1ps[:])
                t1b = work.tile([1, RSHARD], F32, name="t1b")
                nc.sync.dma_start(t1b[:], t1c[1:2, RSHARD:2 * RSHARD])
                t1row = work.tile([1, RSHARD], F32, name="t1row")
                nc.vector.tensor_add(t1row[:], t1c[0:1, 0:RSHARD], t1b[:])

                ttpa = pst.tile([P, 1], F32, name="ttpa", tag="tt", bufs=2)
                nc.tensor.transpose(ttpa[:], t1row[:, 0:P], ident[0:1, 0:1])
                ttpb = pst.tile([64, 1], F32, name="ttpb", tag="tt", bufs=2)
                nc.tensor.transpose(ttpb[:], t1row[:, P:RSHARD],
                                    ident[0:1, 0:1])
                t1sa = work.tile([P, 1], F16, name="t1sa")
                nc.scalar.activation(t1sa[:], ttpa[:], AF.Relu,
                                     scale=1.0 / (SA * SX), bias=lb[:, 0:1])
                t1sb = work.tile([64, 1], F16, name="t1sb")
                nc.scalar.activation(t1sb[:], ttpb[:], AF.Relu,
                                     scale=1.0 / (SA * SX), bias=lb[0:64, 1:2])

                z2 = work.tile([1, 768], F32, name="z2")
                for half in range(2):
                    ps2 = pst.tile([1, 384], F32, name=f"ps2_{half}",
                                   tag="ps2", bufs=2)
                    nc.tensor.matmul(ps2[:], t1sa[:],
                                     l2s[:, half * 384:half * 384 + 384],
                                     start=True, stop=False)
                    nc.tensor.matmul(
                        ps2[:], t1sb[:],
                        l2s[0:64, 768 + half * 384:768 + half * 384 + 384],
                        start=False, stop=True)
                    nc.vector.tensor_copy(z2[:, ts(half, 384)], ps2[:])

                rr_in = dram.tile([1, 768], F32, name="rr_in")
                rr_out = dram.tile([1, 768], F32, name="rr_out",
                                   addr_space="Shared")
                nc.sync.dma_start(rr_in[:], z2[:])
                nc.gpsimd.collective_compute(
                    "AllReduce", ALU.add,
                    replica_groups=[list(range(NCORES))],
                    ins=[rr_in.opt()], outs=[rr_out.opt()])

                # ---- tail on [6, 128]: sigmoid(z+l2b), l3w.t2 + l3b ----
                zz = work.tile([6, P], F32, name="zz")
                nc.sync.dma_start(
                    zz[:], rr_out.rearrange("a (h o) -> (a h) o", o=P))
                nc.vector.tensor_add(zz[:], zz[:], c32r[:, 0:128])
                t2 = work.tile([6, P], F32, name="t2")
                nc.scalar.activation(t2[:], zz[:], AF.Sigmoid)
                p3 = work.tile([6, P], F32, name="p3")
                nc.vector.tensor_mul(p3[:], t2[:], c32r[:, 128:256])
                o6 = work.tile([6, 1], F32, name="o6")
                nc.vector.reduce_sum(o6[:], p3[:], axis=mybir.AxisListType.X)
                oo = work.tile([6, 1], F32, name="oo")
                nc.vector.tensor_add(oo[:], o6[:], c32r[:, 256:257])
                nc.sync.dma_start(out_d[:], oo[:])

    nc.compile()
    return nc


def _prep_inputs(inputs):
    f8 = mybir.dt.np(F8)
    x = np.asarray(inputs["x"], dtype=np.float32)
    adj = np.asarray(inputs["adj"])

    def chunked(arr, nch):
        # [nch*P, C] -> [P, nch*C] with block kc at cols [kc*C:(kc+1)*C]
        c = arr.shape[1]
        return arr.reshape(nch, P, c).transpose(1, 0, 2).reshape(P, nch * c)

    c16 = np.zeros((P, C16_COLS), dtype=np.float16)
    c32 = np.zeros((P, C32_COLS), dtype=np.float32)
    for l, (Fin, F, g) in enumerate(LAYERS):
        W = np.asarray(inputs[f"W{l+1}"], dtype=np.float64)
        a = np.asarray(inputs[f"a{l+1}"], dtype=np.float64)
        b = np.asarray(inputs[f"b{l+1}"], dtype=np.float32)
        waug = np.concatenate([W, (W @ a[:F])[:, None]], axis=1)  # [Fin,F+1]
        off = [C16_WAUG1, C16_WAUG2, C16_WAUG3][l]
        if l == 0:
            c16[:, off:off + 516] = chunked(waug, 4).astype(np.float16)
        else:
            c16[0:Fin, off:off + F + 1] = waug.astype(np.float16)
        aS = a[F:]
        i = np.arange(N)
        wm = np.zeros((N, g), dtype=np.float64)
        wm[i, i % g] = aS[i // g]
        woff = [C16_WSEL1, C16_WSEL2, C16_WSEL3][l]
        c16[:, woff:woff + NCH * g] = chunked(wm, NCH).astype(np.float16)
        boff = [C32_B1, C32_B2, C32_B3][l]
        c32[:, boff:boff + F] = np.broadcast_to(b, (P, F))
    c16[:, C16_IDT:C16_IDT + 128] = np.eye(P, dtype=np.float16)
    c16[:, C16_XT:] = chunked(np.ascontiguousarray(x.T), 4).astype(np.float16)
    c32[:, C32_IDENT:C32_IDENT + 128] = np.eye(P, dtype=np.float32)

    # adj8 i-major: adj8[p, i*8 + kc] = adj[i, kc*128+p]
    adjT = (adj.T > 0).astype(np.float32)      # [j, i]
    adj8 = adjT.reshape(NCH, P, N).transpose(1, 2, 0).reshape(P, N * NCH)
    adj8 = adj8.astype(f8)

    l2w = np.asarray(inputs["l2w"], dtype=np.float32)   # [6,128,256]
    l2b = np.asarray(inputs["l2b"], dtype=np.float32)
    l3w = np.asarray(inputs["l3w"], dtype=np.float32)   # [6,1,128]
    l3b = np.asarray(inputs["l3b"], dtype=np.float32)
    c32r = np.zeros((6, 257), dtype=np.float32)
    c32r[:, 0:128] = l2b
    c32r[:, 128:256] = l3w[:, 0, :]
    c32r[:, 256] = l3b.reshape(-1)

    l1w_flat = np.asarray(inputs["l1w"], dtype=np.float32).reshape(1536, 65536)
    l1b_flat = np.asarray(inputs["l1b"], dtype=np.float32).reshape(1536)
    l1w_q = (l1w_flat * SA).astype(f8)

    # t1 index r = h*256 + t contracts only into head h: block-diagonal
    l2big = np.zeros((1536, 768), dtype=np.float32)
    for h in range(6):
        l2big[ts(h, 256), ts(h, 128)] = l2w[h].T        # [256,128]

    # matvec step V eats xf cols {c0, c0+1, c0+16, c0+17},
    # c0 = 32*(V//8) + 2*(V%8); A position V*4 + s*2 + j <- col c0+16s+j
    V = np.arange(KCH // 4)
    c0 = 32 * (V // 8) + 2 * (V % 8)
    perm = np.stack([c0, c0 + 1, c0 + 16, c0 + 17], axis=1).reshape(-1)

    common = dict(C16=c16, ADJ8=adj8, C32=c32, C32R=c32r)
    in_maps = []
    for c in range(NCORES):
        rows = l1w_q[ts(c, RSHARD)]                     # [192, 65536]
        # xf col t = m*64 + f holds k-tile {(m*128+p)*64 + f : p}
        A = rows.reshape(RSHARD, 8, 128, 64)            # [r, m, p, f]
        A = A.transpose(2, 1, 3, 0).reshape(P, KCH, RSHARD)
        A = A[:, perm, :]
        sub = l2big[ts(c, RSHARD)]                      # [192, 768]
        l2sa = sub[0:128]
        l2sb = np.zeros((P, 768), dtype=np.float32)
        l2sb[0:64] = sub[128:192]
        lbv = np.zeros((P, 2), dtype=np.float32)
        lbv[:, 0] = l1b_flat[c * RSHARD:c * RSHARD + 128]
        lbv[0:64, 1] = l1b_flat[c * RSHARD + 128:(c + 1) * RSHARD]
        m = dict(common)
        m["A8"] = np.ascontiguousarray(A)
        m["L2S"] = np.concatenate([l2sa, l2sb], axis=1).astype(np.float16)
        m["LB"] = lbv
        in_maps.append(m)
    return in_maps


def _ensure_ntff_hook():
    """Register the axon NTFF profile hook (the image's antenv lacks
    axon_hooks; supply it in sys.modules so bass_utils can trace)."""
    try:
        import types

        import antenv
        if "antenv.axon_hooks" not in sys.modules:
            mod = types.ModuleType("antenv.axon_hooks")
            mod._hook = None

            def _set(h, _m=mod):
                _m._hook = h

            def _get(_m=mod):
                return _m._hook

            mod.set_axon_ntff_profile_hook = _set
            mod.get_axon_ntff_profile_hook = _get
            sys.modules["antenv.axon_hooks"] = mod
            antenv.axon_hooks = mod
        from antenv.axon_hooks import (get_axon_ntff_profile_hook,
                                       set_axon_ntff_profile_hook)
        if get_axon_ntff_profile_hook() is None:
            from trn_agent_boot.trn_boot import _ntff_profile_via_ctypes
            set_axon_ntff_profile_hook(
                _ntff_profile_via_ctypes("/opt/axon/libaxon_pjrt.so"))
        return True
    except Exception as e:  # pragma: no cover - profiling is best-effort
        print(f"ntff hook unavailable: {e}", file=sys.stderr)
        return False


def kernel(**inputs) -> np.ndarray:
    if "nc" not in _CACHE:
        _CACHE["nc"] = _build()
    nc = _CACHE["nc"]
    in_maps = _prep_inputs(inputs)
    trace = bool(int(os.environ.get("BASS_KERNEL_TRACE", "0")))
    if trace:
        trace = _ensure_ntff_hook()
    res = run_bass_kernel_spmd(nc, in_maps, list(range(NCORES)), trace=trace)
    _CACHE["last_results"] = res
    return np.asarray(res.results[0]["out"],
                      dtype=np.float32).reshape(6, 1)
